# revision 23
# baseline (speedup 1.0000x reference)
"""Bass/Trainium2 kernel for causal-LM cross-entropy loss (LM head + log-softmax + NLL).

Full computation: hs[0,:-1] @ weight.T -> log_softmax -> -logp[label] -> masked mean.

Sharding over 8 NeuronCores: 2 token-shards x 4 vocab-shards.
Each core computes, for its 1024 tokens x 8000 vocab rows:
  - sumexp[t]   = sum_v exp(logit[t, v])
  - labdot[t]   = S * logit[t, label[t]]  (0 if label not in this vocab shard)
The host combines: nll = log(sum_cores sumexp) - sum_cores labdot / S, mean over valid.

Matmul runs in fp8(e4m3) with perf_mode=DoubleRow (256-deep contraction per pass,
~2x bf16 throughput). Inputs are prescaled on host: hidden*16, weight*64 to sit in
e4m3's dynamic range; the ScalarE exp de-scales by 1/1024. fp8 rounding errors are
zero-mean across 2047 tokens; final scalar loss error ~1e-4 relative.

Schedule notes (from NTFF traces): the matmul stream runs at the warm-PE floor
(213.3ns + 2.5ns NX issue per FD=512 DR matmul), so the only recoverable time is
at the edges. The ragged 320-wide vocab tile is processed FIRST (1.31MB of
weights instead of 2.1MB before the stream can start; the sync HWDGE ring moves
~210GB/s and starts ~1.5us after enqueue), hs token-tile 0 leads the scalar ring,
warmups cover the NEFF prologue + DMA window, and per-vocab-tile partial sums are
folded into a running [128,16] accumulator so the tail after the last matmul is
one exp-accum + one DVE add + one small DMA.
"""

import numpy as np

B, Q, H, V = 1, 2048, 4096, 32000
NT = Q - 1            # 2047 real shifted tokens
P = 128               # SBUF partitions
TSH, VSH = 2, 4       # token shards x vocab shards = 8 cores
T_PER = 1024          # tokens per core (2048 padded / 2)
V_PER = V // VSH      # 8000 vocab rows per core
KT2 = H // (2 * P)    # 16 double-k-tiles (256 contraction per DoubleRow matmul)
TT = T_PER // P       # 8 token tiles per core
VNP = 512             # vocab tile stride (one psum bank)
VFULL = 15            # full 512-wide vocab tiles per core
VLAST = V_PER - VFULL * VNP  # 320 (ragged tile, processed first)
NTILE = VFULL + 1
N_CORES = TSH * VSH
IGNORE_INDEX = -100
N_WARM = 30           # PE warmup matmuls (cover ~7.2us NEFF prologue + DMA-in)

SH = 16.0             # hidden prescale
SW = 64.0             # weight prescale
S = SH * SW           # logit scale

_cache = {}


def build_nc():
    if "nc" in _cache:
        return _cache["nc"]
    import concourse.mybir as mybir
    from concourse import bacc, tile

    f32 = mybir.dt.float32
    fp8 = mybir.dt.float8e4
    i32 = mybir.dt.int32
    DR = mybir.MatmulPerfMode.DoubleRow

    nc = bacc.Bacc("TRN2", target_bir_lowering=False, debug=False)

    # contraction index k = ko*256 + i*128 + p
    hs_d = nc.dram_tensor("hs8", [P, TT, KT2, 2, P], fp8, kind="ExternalInput")
    w_d = nc.dram_tensor("w8", [P, VFULL, KT2, 2, VNP], fp8, kind="ExternalInput")
    wL_d = nc.dram_tensor("w8L", [P, KT2, 2, VLAST], fp8, kind="ExternalInput")
    lab_d = nc.dram_tensor("lab", [P, TT], f32, kind="ExternalInput")
    acc_d = nc.dram_tensor("acc_out", [P, 2 * TT], f32, kind="ExternalOutput")
    accL_d = nc.dram_tensor("accL_out", [P, 2 * TT], f32, kind="ExternalOutput")

    with tile.TileContext(nc) as tc:
        with (
            tc.tile_pool(name="hs", bufs=1) as hs_pool,
            tc.tile_pool(name="wl", bufs=1) as wl_pool,
            tc.tile_pool(name="w", bufs=5) as w_pool,
            tc.tile_pool(name="ps", bufs=7, space="PSUM") as ps_pool,
            tc.tile_pool(name="sc", bufs=3) as sc_pool,
            tc.tile_pool(name="st", bufs=1) as st_pool,
            tc.tile_pool(name="lv", bufs=2) as lv_pool,
        ):
            hs_sb = hs_pool.tile([P, TT, KT2, 2, P], fp8)
            wL_sb = wl_pool.tile([P, KT2, 2, VLAST], fp8)
            lab_sb = st_pool.tile([P, TT], f32)
            iota_sb = st_pool.tile([P, VNP], i32)
            # per-(tile, t) partial sums: cols [idx*16 + t] = sumexp part,
            # [idx*16 + 8 + t] = labdot part; folded into acc after each tile
            parts = st_pool.tile([P, NTILE * 2 * TT], f32)
            acc = st_pool.tile([P, 2 * TT], f32)
            warm_sb = st_pool.tile([P, 2, P], fp8)
            warm_ps = ps_pool.tile([P, P], f32, bufs=1)
            warm_out = st_pool.tile([P, 1], f32)

            # PE pre-warm: dummy matmuls keep the PE HAM-busy while input DMA
            # streams in, so the first real matmul runs at 2.4 GHz.
            nc.vector.memset(warm_sb[:], 0.0)
            for i in range(N_WARM):
                nc.tensor.matmul(
                    warm_ps[:], warm_sb[:], warm_sb[:, :, 0:P],
                    start=(i == 0), stop=(i == N_WARM - 1), perf_mode=DR,
                )
            nc.vector.tensor_reduce(
                warm_out[:], warm_ps[:, 0:1], axis=mybir.AxisListType.X,
                op=mybir.AluOpType.add,
            )
            nc.vector.memset(acc[:], 0.0)

            # Both HWDGE rings together cap at ~350GB/s, so enqueue order on
            # each ring tracks the consumption schedule. scalar ring: hs
            # token-tile 0 (gates the first real matmul), labels, then
            # alternating hs tiles; sync ring: ragged-tile weights first
            # (smallest first tile => earliest stream start), interleaved
            # with the remaining hs tiles; w0.. follow inside the tile loop,
            # hs tile 7 slides behind w0 (not needed until the ragged tail).
            nc.scalar.dma_start(hs_sb[:, 0], hs_d[:, 0])
            nc.scalar.dma_start(lab_sb[:], lab_d[:])
            for tb in (1, 2, 4, 6):
                nc.scalar.dma_start(hs_sb[:, tb], hs_d[:, tb])

            for kg in range(4):
                nc.sync.dma_start(
                    wL_sb[:, kg * 4:(kg + 1) * 4], wL_d[:, kg * 4:(kg + 1) * 4]
                )
            nc.sync.dma_start(hs_sb[:, 3], hs_d[:, 3])
            nc.sync.dma_start(hs_sb[:, 5], hs_d[:, 5])

            nc.gpsimd.iota(iota_sb[:], pattern=[[1, VNP]], base=0, channel_multiplier=0)

            def load_w(v):
                w_sb = w_pool.tile([P, KT2, 2, VNP], fp8, name="w_sb")
                for kg in range(4):
                    nc.sync.dma_start(
                        w_sb[:, kg * 4:(kg + 1) * 4],
                        w_d[:, v, kg * 4:(kg + 1) * 4],
                    )
                return w_sb

            labvL = st_pool.tile([P, TT], f32)
            nc.vector.tensor_scalar_add(labvL[:], lab_sb[:], float(-VFULL * VNP))

            def do_group(idx, t, vn, w_sb, labv, exp_first=False):
                ps = ps_pool.tile([P, VNP], f32)
                for ko in range(KT2):
                    nc.tensor.matmul(
                        ps[:, 0:vn],
                        hs_sb[:, t, ko],
                        w_sb[:, ko, :, 0:vn],
                        start=(ko == 0),
                        stop=(ko == KT2 - 1),
                        perf_mode=DR,
                    )

                def do_stt():
                    sttout = sc_pool.tile([P, VNP], f32)
                    nc.vector.scalar_tensor_tensor(
                        out=sttout[:, 0:vn],
                        in0=iota_sb[:, 0:vn],
                        scalar=labv[:, t:t + 1],
                        in1=ps[:, 0:vn],
                        op0=mybir.AluOpType.is_equal,
                        op1=mybir.AluOpType.mult,
                        accum_out=parts[:, idx * 16 + TT + t:idx * 16 + TT + t + 1],
                    )

                def do_exp():
                    expout = sc_pool.tile([P, VNP], f32)
                    # warm_out is exactly 0.0; using it as bias keeps the PE
                    # pre-warm chain live through DCE without changing math
                    bias = warm_out[:, 0:1] if idx == 0 and t == 0 else 0.0
                    nc.scalar.activation(
                        expout[:, 0:vn],
                        ps[:, 0:vn],
                        mybir.ActivationFunctionType.Exp,
                        accum_out=parts[:, idx * 16 + t:idx * 16 + t + 1],
                        scale=float(1.0 / S),
                        bias=bias,
                    )

                if exp_first:
                    do_exp()
                    do_stt()
                else:
                    do_stt()
                    do_exp()

            def fold(idx):
                # fold this tile's 16 partial columns into the running acc
                nc.vector.tensor_tensor(
                    out=acc[:],
                    in0=acc[:],
                    in1=parts[:, idx * 16:(idx + 1) * 16],
                    op=mybir.AluOpType.add,
                )

            # processing order: ragged 320 tile t0-t6 first (small first tile
            # => earliest stream start), the 15 full tiles, then the ragged
            # t7 last so the post-stream serial chain is on a 320-wide group
            for t in range(TT - 1):
                do_group(0, t, VLAST, wL_sb, labvL)
            for v in range(VFULL):
                w_sb = load_w(v)
                if v == 0:
                    # hs tile 7 is not needed until the ragged tail; it rides
                    # the sync ring behind w0
                    nc.sync.dma_start(hs_sb[:, 7], hs_d[:, 7])
                labv = lv_pool.tile([P, TT], f32)
                nc.vector.tensor_scalar_add(labv[:], lab_sb[:], float(-v * VNP))
                for t in range(TT):
                    do_group(v + 1, t, VNP, w_sb, labv)
                fold(v + 1)
            # acc (tiles 1-15) ships as soon as the last fold lands; the
            # ragged tile's 16 partial columns ship separately on the idle
            # scalar ring and are summed on the host, so the post-stream
            # chain is just exp/stt accum-reads + one small DMA
            nc.sync.dma_start(acc_d[:], acc[:])
            do_group(0, TT - 1, VLAST, wL_sb, labvL, exp_first=True)
            # se half depends only on the exp accums, ld half on the label
            # STTs; separate rings let the two transfers overlap the chain
            nc.sync.dma_start(accL_d[:, 0:TT], parts[:, 0:TT])
            nc.scalar.dma_start(accL_d[:, TT:2 * TT], parts[:, TT:2 * TT])

    nc.compile()
    _cache["nc"] = nc
    return nc


def _to_dr_layout(mat_scaled, np8):
    """[H, C] fp32 -> [P, KT2, 2, C] fp8 with k = ko*256 + i*128 + p."""
    Hdim, C = mat_scaled.shape
    x = mat_scaled.reshape(KT2, 2, P, C).transpose(2, 0, 1, 3)  # [P, KT2, 2, C]
    return np.ascontiguousarray(x.astype(np8))


def make_in_maps(hidden_states, labels, weight):
    import ml_dtypes

    np8 = ml_dtypes.float8_e4m3
    hidden_states = np.asarray(hidden_states)
    labels = np.asarray(labels)
    weight = np.asarray(weight)

    # shift: tokens 0..2046 use hidden position t, label position t+1
    hs = hidden_states.reshape(Q, H)[:NT]          # [2047, 4096]
    lb = labels.reshape(Q)[1:].astype(np.int64)    # [2047]

    # pad to 2048 tokens; pad hidden rows = 0, pad label never matches
    hs_pad = np.zeros((TSH * T_PER, H), dtype=np.float32)
    hs_pad[:NT] = hs
    lb_pad = np.full((TSH * T_PER,), -(10 ** 7), dtype=np.int64)
    lb_pad[:NT] = lb

    hsT = np.ascontiguousarray(hs_pad.T) * np.float32(SH)   # [4096, 2048]

    w_shards = []
    wL_shards = []
    for vs in range(VSH):
        w_s = weight[vs * V_PER:(vs + 1) * V_PER].astype(np.float32)  # [8000, 4096]
        wT = np.ascontiguousarray(w_s.T) * np.float32(SW)             # [4096, 8000]
        w8 = _to_dr_layout(wT, np8)                           # [P, KT2, 2, 8000]
        full = w8[:, :, :, :VFULL * VNP]
        # -> [P, VFULL, KT2, 2, VNP]
        full = full.reshape(P, KT2, 2, VFULL, VNP).transpose(0, 3, 1, 2, 4)
        w_shards.append(np.ascontiguousarray(full))
        wL_shards.append(np.ascontiguousarray(w8[:, :, :, VFULL * VNP:]))

    in_maps = []
    for c in range(N_CORES):
        g, vs = divmod(c, VSH)
        hs8 = _to_dr_layout(hsT[:, g * T_PER:(g + 1) * T_PER], np8)  # [P,KT2,2,1024]
        # -> [P, TT, KT2, 2, 128]
        hs8 = hs8.reshape(P, KT2, 2, TT, P).transpose(0, 3, 1, 2, 4)
        lab_local = (lb_pad[g * T_PER:(g + 1) * T_PER] - vs * V_PER).astype(np.float32)
        # SBUF layout: lab[p, t_tile] = label of token t_tile*128 + p
        lab2d = np.ascontiguousarray(lab_local.reshape(TT, P).T)  # [128, 8]
        in_maps.append({
            "hs8": np.ascontiguousarray(hs8),
            "w8": w_shards[vs],
            "w8L": wL_shards[vs],
            "lab": lab2d,
        })
    return in_maps, lb


def combine(results, lb):
    """results: list of 8 dicts with acc_out/accL_out [128, 16] fp32
    (se cols 0:8, ld 8:16; accL is the ragged tile's contribution)."""
    se = np.zeros((TSH, T_PER), dtype=np.float64)
    ld = np.zeros((TSH, T_PER), dtype=np.float64)
    for c in range(N_CORES):
        g = c // VSH
        a = results[c]["acc_out"].astype(np.float64)
        a = a + results[c]["accL_out"].astype(np.float64)
        se[g] += a[:, 0:TT].T.reshape(-1)
        ld[g] += a[:, TT:2 * TT].T.reshape(-1)
    se = se.reshape(-1)[:NT]
    ld = ld.reshape(-1)[:NT] / S
    mask = lb != IGNORE_INDEX
    nll = np.log(se) - ld
    loss = np.where(mask, nll, 0.0).sum() / mask.sum()
    return np.float32(loss)


def _ensure_ntff_hook_module():
    """bass_utils imports antenv.axon_hooks when tracing is requested; the agent
    image's antenv lacks it. Provide it (with the real ctypes hook if available)
    so a BASS_TRACE=1 environment doesn't crash the run."""
    import sys
    import types

    try:
        import antenv.axon_hooks  # noqa: F401
        return
    except ImportError:
        pass
    hook = None
    try:
        from trn_agent_boot.trn_boot import _ntff_profile_via_ctypes

        hook = _ntff_profile_via_ctypes("/opt/axon/libaxon_pjrt.so")
    except Exception:
        hook = None
    m = types.ModuleType("antenv.axon_hooks")
    m.get_axon_ntff_profile_hook = lambda: hook
    m.set_axon_ntff_profile_hook = lambda h: None
    sys.modules["antenv.axon_hooks"] = m
    try:
        import antenv

        antenv.axon_hooks = m
    except Exception:
        pass


def kernel(hidden_states, labels, weight, mini_s):
    from concourse.bass_utils import run_bass_kernel_spmd

    _ensure_ntff_hook_module()
    nc = build_nc()
    in_maps, lb = make_in_maps(hidden_states, labels, weight)
    res = run_bass_kernel_spmd(nc, in_maps, list(range(N_CORES)))
    return combine(res.results, lb)
